# revision 4
# baseline (speedup 1.0000x reference)
"""RGCN (2-layer, basis decomposition) link-predict encoder on 8 Trainium2 cores.

v6 design (SBUF-resident features+messages, ap_gather, indicator matmuls):
  - Host: W_r = sum_b w_comp[r,b] bases[b]; edges sharded by dst block.
  - H^T (features transposed, bf16 [128, N]) resident in SBUF one half at a
    time; phase-1 gathers h[src]^T columns with gpsimd.ap_gather at 4-byte
    (adjacent node pair) granularity; edges grouped by
    (half, etype, parity-of-src) so a stride-2 slice of the gathered pair
    buffer is the matmul rhs.
  - MM1: lhsT = W_r (stationary per etype), rhs = X^T -> M^T [dout, e] in
    PSUM; batched ScalarE copies move 8-tile PSUM regions into an SBUF
    message-chunk pool (messages never touch DRAM). Phase-1/phase-2 are
    interleaved chunk by chunk over a single message buffer.
  - Bridge: ap_gather (d=2 pairs) permutes message columns from etype order
    to dst-window order; (group, window) runs are padded to even length so
    pairs never straddle windows.
  - Scatter: PE-transpose bridge tiles (batched into PSUM banks), DVE
    batched copy to SBUF, then matmul against host-precomputed norm-weighted
    indicator tiles (IndN, streamed from DRAM). Layer 1 accumulates out^T
    [dout, slot] (feeds h1^T directly); layer 2 accumulates out rows.
  - Self-loop via W_loop matmul against streamed h_blk^T; bias via K=1
    matmul; AllGather of h1^T bf16 between layers.
"""

import os
import sys
import numpy as np

for _p in ("/opt/trn_rl_repo", "/root/.axon_site/_ro/trn_rl_repo"):
    if os.path.isdir(_p) and _p not in sys.path:
        sys.path.append(_p)

import ml_dtypes
import concourse.bass as bass
import concourse.mybir as mybir
import concourse.tile as tile
import concourse.bacc as bacc
from concourse.bass_utils import run_bass_kernel_spmd

P = 128
HALF = 25088     # nodes per H^T half (pairs: 12544)
GB1 = 12         # phase-1 ap_gather batch (tiles per call)
GB2 = 24         # phase-2 bridge ap_gather batch (tiles per call)
CPY = 8          # tiles per batched PSUM->SBUF copy / IndN load
N_CHUNKS = 6


def _ceil_div(a, b):
    return (a + b - 1) // b


def _wrap_idx16(flat):
    """[n] int -> [128, n//16] int16: idx i at [i%16, i//16], replicated
    across the 8 q7 cores' 16-partition stripes."""
    n = len(flat)
    assert n % 16 == 0
    a = np.asarray(flat, np.int16).reshape(n // 16, 16)
    a = np.ascontiguousarray(a.T)
    return np.ascontiguousarray(np.tile(a, (8, 1)))


def _preprocess(src, dst, etype, norm, n_nodes, n_rels, n_cores):
    NB = n_nodes // n_cores
    NW = _ceil_div(NB, P)
    NG = 2 * n_rels * 2     # (half, etype, parity)

    src = np.asarray(src, np.int64)
    dst = np.asarray(dst, np.int64)
    etype = np.asarray(etype, np.int64)
    norm = np.asarray(norm, np.float32).reshape(-1)

    # ---- per-core grouping ----
    cores = []
    pad_gw_all = np.zeros((n_cores, NG, NW), np.int64)
    for c in range(n_cores):
        m = (dst // NB) == c
        es, ed, ee, en = src[m], dst[m], etype[m], norm[m]
        g = (es >= HALF) * (2 * n_rels) + ee * 2 + (es % 2)
        w = (ed - c * NB) // P
        cnt_gw = np.zeros((NG, NW), np.int64)
        np.add.at(cnt_gw, (g, w), 1)
        pad_gw_all[c] = ((cnt_gw + 1) // 2) * 2
        cores.append((es, ed, ee, en, g, w))

    S_g = pad_gw_all.sum(2)                      # [cores, NG]
    T_g = np.array([_ceil_div(int(S_g[:, g].max()), P) for g in range(NG)])
    base_g = np.concatenate([[0], np.cumsum(T_g)])
    T1 = int(base_g[-1])

    # chunks = contiguous whole groups
    tgt = _ceil_div(T1, N_CHUNKS)
    chunk_of_g = np.zeros(NG, np.int64)
    ck, used = 0, 0
    for g in range(NG):
        if used >= tgt and ck < N_CHUNKS - 1:
            ck, used = ck + 1, 0
        chunk_of_g[g] = ck
        used += T_g[g]
    n_chunks = int(chunk_of_g.max()) + 1
    chunk_t0 = [int(base_g[int(np.argmax(chunk_of_g == k))])
                for k in range(n_chunks)]
    chunk_t1 = chunk_t0[1:] + [T1]
    chunk_tiles = [chunk_t1[k] - chunk_t0[k] for k in range(n_chunks)]

    half_of_tile = np.zeros(T1, np.int64)
    rel_of_tile = np.zeros(T1, np.int64)
    par_of_tile = np.zeros(T1, np.int64)
    for g in range(NG):
        half_of_tile[base_g[g]:base_g[g + 1]] = g // (2 * n_rels)
        rel_of_tile[base_g[g]:base_g[g + 1]] = (g % (2 * n_rels)) // 2
        par_of_tile[base_g[g]:base_g[g + 1]] = g % 2

    # phase-1 calls: per chunk, contiguous tile runs within one half
    p1_calls = []       # (chunk, half, t0, ktiles)
    for k in range(n_chunks):
        t = chunk_t0[k]
        while t < chunk_t1[k]:
            kk = 1
            while (kk < GB1 and t + kk < chunk_t1[k]
                   and half_of_tile[t + kk] == half_of_tile[t]):
                kk += 1
            p1_calls.append((k, int(half_of_tile[t]), t, kk))
            t += kk

    # ---- phase-2 layout ----
    cnt_cw = np.zeros((n_cores, n_chunks, NW), np.int64)
    for c in range(n_cores):
        for g in range(NG):
            cnt_cw[c, chunk_of_g[g]] += pad_gw_all[c, g]
    T_cw = np.zeros((n_chunks, NW), np.int64)
    for k in range(n_chunks):
        for w in range(NW):
            T_cw[k, w] = _ceil_div(int(cnt_cw[:, k, w].max()), P)
    base_cw = np.concatenate([[0], np.cumsum(T_cw.reshape(-1))])
    T2 = int(base_cw[-1])

    p2_calls = []       # (chunk, t0, ktiles)
    for k in range(n_chunks):
        t0 = int(base_cw[k * NW])
        t1 = int(base_cw[(k + 1) * NW]) if k + 1 < n_chunks else T2
        t = t0
        while t < t1:
            kk = min(GB2, t1 - t)
            p2_calls.append((k, t, kk))
            t += kk

    # ---- per-core tables (vectorized) ----
    per_core = []
    for c in range(n_cores):
        es, ed, ee, en, g, w = cores[c]
        ne = len(es)
        pad_gw = pad_gw_all[c]

        # run starts in phase-1 slot space
        run_start1 = np.zeros((NG, NW), np.int64)
        for gg in range(NG):
            starts = base_g[gg] * P + np.concatenate(
                [[0], np.cumsum(pad_gw[gg])[:-1]])
            run_start1[gg] = starts
        # run starts in phase-2 slot space
        run_start2 = np.zeros((NG, NW), np.int64)
        for k in range(n_chunks):
            gsel = np.where(chunk_of_g == k)[0]
            for w_ in range(NW):
                p0 = base_cw[k * NW + w_] * P
                starts = p0 + np.concatenate(
                    [[0], np.cumsum(pad_gw[gsel, w_])[:-1]])
                run_start2[gsel, w_] = starts

        order = np.lexsort((w, g))
        gs, ws = g[order], w[order]
        key = gs * NW + ws
        # offset within run
        first = np.concatenate([[True], key[1:] != key[:-1]])
        runid = np.cumsum(first) - 1
        firstpos = np.nonzero(first)[0]
        off = np.arange(ne) - firstpos[runid]
        slot1 = np.empty(ne, np.int64)
        slot2 = np.empty(ne, np.int64)
        slot1[order] = run_start1[gs, ws] + off
        slot2[order] = run_start2[gs, ws] + off

        p1_pair = np.zeros(T1 * P, np.int64)
        pairloc = np.where(es < HALF, es // 2, (es - HALF) // 2)
        p1_pair[slot1] = pairloc

        p2_pair = np.zeros(T2 * P // 2, np.int64)
        # pair k2 of p2 gathers p1 pair: runs even-aligned in both spaces
        # -> p1slot = s1 + off, p2slot = s2 + off with s1, s2 even
        ck_of_slot1 = np.searchsorted(
            np.asarray(chunk_t0[1:]) * P, slot1, "right")
        cpb = np.asarray([chunk_t0[k] * (P // 2) for k in range(n_chunks)])
        even = (off % 2 == 0)
        src_pair = slot1 // 2 - cpb[ck_of_slot1]
        m_even = np.zeros(ne, bool)
        m_even[order] = even
        p2_pair[slot2[m_even] // 2] = src_pair[m_even]
        # odd member of each pair maps automatically (same pair)

        indn = np.zeros((P, T2 * P), np.float32)
        loc = (ed - c * NB) % P
        indn[slot2 % P, (slot2 // P) * P + loc] = en

        per_core.append(dict(
            p1i=_wrap_idx16(p1_pair),
            p2i=_wrap_idx16(p2_pair),
            indn=np.ascontiguousarray(indn.astype(ml_dtypes.bfloat16)),
        ))

    struct = dict(
        NB=NB, NW=NW, T1=T1, T2=T2,
        n_chunks=n_chunks, chunk_t0=[int(x) for x in chunk_t0],
        chunk_tiles=[int(x) for x in chunk_tiles],
        base_cw=[int(x) for x in base_cw],
        T_cw=[[int(T_cw[k, w]) for w in range(NW)] for k in range(n_chunks)],
        rel_of_tile=[int(x) for x in rel_of_tile],
        par_of_tile=[int(x) for x in par_of_tile],
        half_of_tile=[int(x) for x in half_of_tile],
        p1_calls=p1_calls, p2_calls=p2_calls,
        n_rels=n_rels, n_cores=n_cores,
    )
    return struct, per_core


def _build_program(struct, n_nodes, d):
    NB, NW = struct["NB"], struct["NW"]
    T1, T2 = struct["T1"], struct["T2"]
    n_chunks = struct["n_chunks"]
    chunk_t0 = struct["chunk_t0"]
    chunk_tiles = struct["chunk_tiles"]
    base_cw, T_cw = struct["base_cw"], struct["T_cw"]
    rel_of_tile = struct["rel_of_tile"]
    par_of_tile = struct["par_of_tile"]
    p1_calls, p2_calls = struct["p1_calls"], struct["p2_calls"]
    n_rels = struct["n_rels"]
    n_cores = struct["n_cores"]
    f32, bf16, i16 = mybir.dt.float32, mybir.dt.bfloat16, mybir.dt.int16
    Act = mybir.ActivationFunctionType
    NGW = n_rels + 1
    HCOLS = 2 * HALF
    MCT = max(chunk_tiles)

    nc = bacc.Bacc("TRN2", target_bir_lowering=False, debug=False,
                   num_devices=n_cores)

    h0T = nc.dram_tensor("h0T", [P, HCOLS], bf16, kind="ExternalInput")
    hblkT1 = nc.dram_tensor("hblkT1", [P, NW * P], bf16, kind="ExternalInput")
    w1 = nc.dram_tensor("w1", [P, NGW * d], bf16, kind="ExternalInput")
    w2 = nc.dram_tensor("w2", [P, NGW * d], bf16, kind="ExternalInput")
    b1 = nc.dram_tensor("b1", [1, d], bf16, kind="ExternalInput")
    b2 = nc.dram_tensor("b2", [1, d], bf16, kind="ExternalInput")
    p1i = nc.dram_tensor("p1i", [P, T1 * 8], i16, kind="ExternalInput")
    p2i = nc.dram_tensor("p2i", [P, T2 * 4], i16, kind="ExternalInput")
    indn = nc.dram_tensor("indn", [P, T2 * P], bf16, kind="ExternalInput")
    out = nc.dram_tensor("out", [NB, d], f32, kind="ExternalOutput")

    h1T = nc.dram_tensor("h1T", [P, NW * P], bf16)
    h1Tfull = nc.dram_tensor("h1Tfull", [n_cores * P, NW * P], bf16)

    with tile.TileContext(nc) as tc:
        with (
            tc.tile_pool(name="cst", bufs=1) as cst,
            tc.tile_pool(name="wp", bufs=1) as wp,
            tc.tile_pool(name="hp", bufs=1) as hp,
            tc.tile_pool(name="msp", bufs=1) as msp,
            tc.tile_pool(name="g1p", bufs=2) as g1p,
            tc.tile_pool(name="g2p", bufs=2) as g2p,
            tc.tile_pool(name="mso", bufs=3) as mso,
            tc.tile_pool(name="inp", bufs=3) as inp,
            tc.tile_pool(name="hbp", bufs=2) as hbp,
            tc.tile_pool(name="accp", bufs=1) as accp,
            tc.tile_pool(name="wbp", bufs=2) as wbp,
            tc.tile_pool(name="ps1", bufs=2, space="PSUM") as ps1,
            tc.tile_pool(name="pst", bufs=2, space="PSUM") as pst,
            tc.tile_pool(name="pso", bufs=2, space="PSUM") as pso,
        ):
            ident = cst.tile([P, P], bf16)
            nc.gpsimd.memset(ident[:], 0.0)
            nc.gpsimd.affine_select(
                out=ident[:], in_=ident[:],
                compare_op=mybir.AluOpType.not_equal, fill=1.0,
                base=0, pattern=[[-1, P]], channel_multiplier=1,
            )
            ones_row = cst.tile([1, P], bf16)
            nc.gpsimd.memset(ones_row[:], 1.0)
            p1i_sb = cst.tile([P, T1 * 8], i16)
            nc.sync.dma_start(p1i_sb[:], p1i[:, :])
            p2i_sb = cst.tile([P, T2 * 4], i16)
            nc.sync.dma_start(p2i_sb[:], p2i[:, :])
            b1_sb = cst.tile([1, d], bf16)
            nc.sync.dma_start(b1_sb[:], b1[:, :])
            b2_sb = cst.tile([1, d], bf16)
            nc.sync.dma_start(b2_sb[:], b2[:, :])

            def layer(h_src_dram, hblkT_dram, w_dram, bias_sb, relu,
                      transposed_out, out_dram):
                w_sb = wp.tile([P, NGW * d], bf16, tag="w")
                nc.sync.dma_start(w_sb[:], w_dram[:, :])

                hh = {}

                def ensure_half(hf):
                    if hf in hh:
                        return hh[hf]
                    ht = hp.tile([P, HALF], bf16, tag="h")
                    if h_src_dram is h0T:
                        nc.sync.dma_start(
                            ht[:], h0T[:, hf * HALF:(hf + 1) * HALF])
                    else:
                        done = 0
                        while done < HALF:
                            v = hf * HALF + done
                            cc = v // NB
                            if cc >= n_cores:
                                nc.gpsimd.memset(ht[:, done:HALF], 0.0)
                                break
                            j = v % NB
                            n = min(NB - j, HALF - done)
                            nc.sync.dma_start(
                                ht[:, done:done + n],
                                h1Tfull[cc * P:(cc + 1) * P, j:j + n])
                            done += n
                    hh.clear()          # only one half resident
                    hh[hf] = ht
                    return ht

                accs = [None] * NW
                p1c = 0
                p2c = 0

                # IndN / hblkT streaming (global CPY-aligned batches)
                ind_of_tile = {}

                def ensure_ind(t):
                    t0 = (t // CPY) * CPY
                    if t0 in ind_of_tile:
                        return
                    nt = min(CPY, T2 - t0)
                    it = inp.tile([P, CPY * P], bf16, tag="ind")
                    nc.sync.dma_start(it[:, :nt * P],
                                      indn[:, t0 * P:(t0 + nt) * P])
                    ind_of_tile[t0] = it

                hb_of_w = {}

                def ensure_hb(w):
                    w0 = (w // 8) * 8
                    if w0 in hb_of_w:
                        return
                    nw_ = min(8, NW - w0)
                    hbt = hbp.tile([P, 8 * P], bf16, tag="hb")
                    nc.sync.dma_start(hbt[:, :nw_ * P],
                                      hblkT_dram[:, w0 * P:(w0 + nw_) * P])
                    hb_of_w[w0] = hbt

                for ck in range(n_chunks):
                    # ---- phase 1 of chunk ck ----
                    msgs = msp.tile([P, MCT * P], bf16, tag="ms")
                    mm1_ps = None
                    mm1_t0 = 0

                    def flush_mm1(t_end):
                        nonlocal mm1_ps, mm1_t0
                        if mm1_ps is None:
                            return
                        nt = t_end - mm1_t0
                        r0 = (mm1_t0 - chunk_t0[ck]) * P
                        nc.scalar.activation(
                            msgs[:, r0:r0 + nt * P], mm1_ps[:, :nt * P],
                            Act.Copy)
                        mm1_ps = None

                    while p1c < len(p1_calls) and p1_calls[p1c][0] == ck:
                        _, hf, t0, ktiles = p1_calls[p1c]
                        p1c += 1
                        ht = ensure_half(hf)
                        xt = g1p.tile([P, GB1 * 2 * P], bf16, tag="g1")
                        nc.gpsimd.ap_gather(
                            out_ap=xt[:, :ktiles * 2 * P].rearrange(
                                "p (n two) -> p n two", two=2),
                            in_ap=ht[:].rearrange(
                                "p (n two) -> p n two", two=2),
                            idxs_ap=p1i_sb[:, t0 * 8:(t0 + ktiles) * 8],
                            channels=P, num_elems=HALF // 2, d=2,
                            num_idxs=ktiles * P,
                        )
                        xt_v = xt[:].rearrange("p (n two) -> p n two", two=2)
                        for kk in range(ktiles):
                            t = t0 + kk
                            if mm1_ps is not None and t - mm1_t0 == CPY:
                                flush_mm1(t)
                            if mm1_ps is None:
                                mm1_ps = ps1.tile([P, CPY * P], f32,
                                                  tag="m1", space="PSUM")
                                mm1_t0 = t
                            r = rel_of_tile[t]
                            q = par_of_tile[t]
                            rhs_ap = xt_v[:, kk * P:(kk + 1) * P, q:q + 1
                                          ].rearrange("p n one -> p (n one)")
                            nc.tensor.matmul(
                                out=mm1_ps[:, (t - mm1_t0) * P
                                           :(t - mm1_t0 + 1) * P],
                                lhsT=w_sb[:, r * d:(r + 1) * d],
                                rhs=rhs_ap,
                                start=True, stop=True,
                            )
                        flush_mm1(t0 + ktiles)

                    # ---- phase 2 of chunk ck ----
                    gather_of_tile = {}

                    def ensure_gathered(t):
                        nonlocal p2c
                        while (t not in gather_of_tile
                               and p2c < len(p2_calls)
                               and p2_calls[p2c][0] == ck):
                            _, t0_, kt_ = p2_calls[p2c]
                            p2c += 1
                            g2 = g2p.tile([P, GB2 * P], bf16, tag="g2")
                            nc.gpsimd.ap_gather(
                                out_ap=g2[:, :kt_ * P].rearrange(
                                    "p (n two) -> p n two", two=2),
                                in_ap=msgs[:, :chunk_tiles[ck] * P].rearrange(
                                    "p (n two) -> p n two", two=2),
                                idxs_ap=p2i_sb[:, t0_ * 4:(t0_ + kt_) * 4],
                                channels=P,
                                num_elems=chunk_tiles[ck] * P // 2,
                                d=2, num_idxs=kt_ * P // 2,
                            )
                            for k in range(kt_):
                                gather_of_tile[t0_ + k] = (g2, k)

                    msort_of_tile = {}

                    def ensure_msort(t, gbase, glim):
                        tb = gbase + ((t - gbase) // CPY) * CPY
                        if tb in msort_of_tile:
                            return
                        nt = min(CPY, glim - tb)
                        tr_ps = pst.tile([P, CPY * P], bf16, tag="tr",
                                         space="PSUM")
                        for k in range(nt):
                            ensure_gathered(tb + k)
                            g2, kk = gather_of_tile[tb + k]
                            nc.tensor.transpose(
                                out=tr_ps[:, k * P:(k + 1) * P],
                                in_=g2[:, kk * P:(kk + 1) * P],
                                identity=ident[:])
                        ms = mso.tile([P, CPY * P], bf16, tag="msrt")
                        nc.vector.tensor_copy(ms[:, :nt * P],
                                              tr_ps[:, :nt * P])
                        msort_of_tile[tb] = ms

                    for w in range(NW):
                        ntl = T_cw[ck][w]
                        has_self = ck == 0
                        if ntl == 0 and not has_self:
                            continue
                        gbase = int(base_cw[ck * NW + w])
                        glim = gbase + ntl
                        o_ps = pso.tile([P, P], f32, tag="op", space="PSUM")
                        n_mm = ntl + (2 if has_self else 0)
                        i_mm = 0
                        for k in range(ntl):
                            t = gbase + k
                            ensure_msort(t, gbase, glim)
                            tb = gbase + ((t - gbase) // CPY) * CPY
                            ms = msort_of_tile[tb]
                            mk = t - tb
                            ensure_ind(t)
                            it = ind_of_tile[(t // CPY) * CPY]
                            ik = t - (t // CPY) * CPY
                            if transposed_out:
                                nc.tensor.matmul(
                                    out=o_ps[:],
                                    lhsT=ms[:, mk * P:(mk + 1) * P],
                                    rhs=it[:, ik * P:(ik + 1) * P],
                                    start=(i_mm == 0),
                                    stop=(i_mm == n_mm - 1))
                            else:
                                nc.tensor.matmul(
                                    out=o_ps[:],
                                    lhsT=it[:, ik * P:(ik + 1) * P],
                                    rhs=ms[:, mk * P:(mk + 1) * P],
                                    start=(i_mm == 0),
                                    stop=(i_mm == n_mm - 1))
                            i_mm += 1
                        if has_self:
                            ensure_hb(w)
                            hbt = hb_of_w[(w // 8) * 8]
                            hk = w - (w // 8) * 8
                            if transposed_out:
                                nc.tensor.matmul(
                                    out=o_ps[:],
                                    lhsT=w_sb[:, n_rels * d:(n_rels + 1) * d],
                                    rhs=hbt[:, hk * P:(hk + 1) * P],
                                    start=(i_mm == 0), stop=False)
                                nc.tensor.matmul(
                                    out=o_ps[:],
                                    lhsT=bias_sb[0:1, :],
                                    rhs=ones_row[0:1, :],
                                    start=False, stop=True)
                            else:
                                nc.tensor.matmul(
                                    out=o_ps[:],
                                    lhsT=hbt[:, hk * P:(hk + 1) * P],
                                    rhs=w_sb[:, n_rels * d:(n_rels + 1) * d],
                                    start=(i_mm == 0), stop=False)
                                nc.tensor.matmul(
                                    out=o_ps[:],
                                    lhsT=ones_row[0:1, :],
                                    rhs=bias_sb[0:1, :],
                                    start=False, stop=True)
                        if accs[w] is None:
                            acc = accp.tile([P, P], f32, tag=f"acc{w}")
                            nc.vector.tensor_copy(acc[:], o_ps[:])
                            accs[w] = acc
                        else:
                            nc.vector.tensor_add(accs[w][:], accs[w][:],
                                                 o_ps[:])

                # ---- epilogue ----
                WB = 8
                wb = None
                wb_w0 = 0
                for w in range(NW):
                    if wb is None:
                        wb = wbp.tile(
                            [P, WB * P], bf16 if transposed_out else f32,
                            tag="wb")
                        wb_w0 = w
                    seg = wb[:, (w - wb_w0) * P:(w - wb_w0 + 1) * P]
                    nc.scalar.activation(seg, accs[w][:],
                                         Act.Relu if relu else Act.Copy)
                    if w - wb_w0 + 1 == WB or w == NW - 1:
                        nw_ = w - wb_w0 + 1
                        if transposed_out:
                            nc.sync.dma_start(
                                out_dram[:, wb_w0 * P:(wb_w0 + nw_) * P],
                                wb[:, :nw_ * P])
                        else:
                            rows = min(nw_ * P, NB - wb_w0 * P)
                            full = (rows // P) * P
                            if full:
                                nc.sync.dma_start(
                                    out_dram[wb_w0 * P:wb_w0 * P + full, :]
                                    .rearrange("(t p) d -> p t d", p=P),
                                    wb[:, :full].rearrange(
                                        "p (t d) -> p t d", d=P))
                            rem = rows - full
                            if rem:
                                nc.sync.dma_start(
                                    out_dram[wb_w0 * P + full
                                             :wb_w0 * P + rows, :],
                                    wb[:rem, full:full + P])
                        wb = None

            layer(h0T, hblkT1, w1, b1_sb, True, True, h1T)
            nc.gpsimd.collective_compute(
                "AllGather", mybir.AluOpType.bypass,
                replica_groups=[list(range(n_cores))],
                ins=[h1T.ap().opt()], outs=[h1Tfull.ap().opt()],
            )
            layer(h1Tfull, h1T, w2, b2_sb, False, False, out)

    nc.finalize()
    return nc


_CACHE = {}


def _get_program(struct, n_nodes, d):
    key = (n_nodes, d, struct["T1"], struct["T2"],
           tuple(struct["rel_of_tile"]), tuple(struct["base_cw"]))
    if key not in _CACHE:
        _CACHE[key] = _build_program(struct, n_nodes, d)
    return _CACHE[key]


def prepare(h_ids, src, dst, etype, norm, embedding,
            w_comp1, bases1, loop_w1, bias1,
            w_comp2, bases2, loop_w2, bias2, n_cores=8):
    h_ids = np.asarray(h_ids).astype(np.int64)
    src = np.asarray(src).astype(np.int64)
    dst = np.asarray(dst).astype(np.int64)
    etype = np.asarray(etype).astype(np.int64)
    norm = np.asarray(norm, dtype=np.float32)
    embedding = np.asarray(embedding, dtype=np.float32)
    n_nodes, d = embedding.shape
    n_rels = np.asarray(w_comp1).shape[0]
    NB = n_nodes // n_cores
    NW = _ceil_div(NB, P)
    HCOLS = 2 * HALF

    W1 = np.einsum("rb,bio->rio", np.asarray(w_comp1, np.float64),
                   np.asarray(bases1, np.float64)).astype(np.float32)
    W2 = np.einsum("rb,bio->rio", np.asarray(w_comp2, np.float64),
                   np.asarray(bases2, np.float64)).astype(np.float32)
    W1 = np.concatenate([W1, np.asarray(loop_w1, np.float32)[None]], 0)
    W2 = np.concatenate([W2, np.asarray(loop_w2, np.float32)[None]], 0)
    w1_dev = np.ascontiguousarray(
        np.transpose(W1, (1, 0, 2)).reshape(d, (n_rels + 1) * d)
    ).astype(ml_dtypes.bfloat16)
    w2_dev = np.ascontiguousarray(
        np.transpose(W2, (1, 0, 2)).reshape(d, (n_rels + 1) * d)
    ).astype(ml_dtypes.bfloat16)
    b1_dev = np.asarray(bias1, np.float32).reshape(1, d).astype(
        ml_dtypes.bfloat16)
    b2_dev = np.asarray(bias2, np.float32).reshape(1, d).astype(
        ml_dtypes.bfloat16)

    h0 = embedding[h_ids].astype(ml_dtypes.bfloat16)
    h0T = np.zeros((P, HCOLS), ml_dtypes.bfloat16)
    h0T[:, :n_nodes] = np.ascontiguousarray(h0.T)

    struct, per_core = _preprocess(src, dst, etype, norm, n_nodes, n_rels,
                                   n_cores)

    in_maps = []
    for c in range(n_cores):
        pc = per_core[c]
        hblkT1 = np.zeros((P, NW * P), ml_dtypes.bfloat16)
        hblkT1[:, :NB] = h0T[:, c * NB:(c + 1) * NB]
        in_maps.append({
            "h0T": h0T, "hblkT1": hblkT1,
            "w1": w1_dev, "w2": w2_dev, "b1": b1_dev, "b2": b2_dev,
            "p1i": pc["p1i"], "p2i": pc["p2i"], "indn": pc["indn"],
        })
    return struct, in_maps, n_nodes, d


def run(h_ids, src, dst, etype, norm, embedding,
        w_comp1, bases1, loop_w1, bias1,
        w_comp2, bases2, loop_w2, bias2,
        n_cores=8, trace=False):
    struct, in_maps, n_nodes, d = prepare(
        h_ids, src, dst, etype, norm, embedding,
        w_comp1, bases1, loop_w1, bias1,
        w_comp2, bases2, loop_w2, bias2, n_cores)
    nc = _get_program(struct, n_nodes, d)
    res = run_bass_kernel_spmd(
        nc, in_maps, core_ids=list(range(n_cores)), trace=trace)
    blocks = [res.results[c]["out"] for c in range(n_cores)]
    full = np.concatenate(blocks, 0)[:n_nodes]
    if trace:
        return full, res
    return full


def kernel(h_ids, src, dst, etype, norm, embedding,
           w_comp1, bases1, loop_w1, bias1,
           w_comp2, bases2, loop_w2, bias2):
    return run(h_ids, src, dst, etype, norm, embedding,
               w_comp1, bases1, loop_w1, bias1,
               w_comp2, bases2, loop_w2, bias2)


# revision 13
# speedup vs baseline: 3.9339x; 3.9339x over previous
"""RGCN (2-layer, basis decomposition) link-predict encoder on 8 Trainium2 cores.

v7 design (SBUF-resident features+messages, queue-parallel SWDGE gathers):
  - Host: W_r = sum_b w_comp[r,b] bases[b]; edges sharded by dst block.
    Self-loops are appended as ordinary edges (etype = n_rels, norm = 1).
  - Features live in DRAM in a "wrapped padded-block" layout: token
    t = core*6272 + local_row sits at [t%128, t//128] as a 256B row. Each
    layer stages one 25088-token half of the table into SBUF and phase 1
    gathers h[src]^T tiles with SBUF-source dma_gather (transpose); calls
    round-robin over 4 SWDGE queues, which run on distinct Q7 core pairs
    and therefore generate descriptors concurrently.
  - MM1 per tile: lhsT = gathered X^T, rhs = W_r -> M rows [e, dout] in
    batched PSUM regions; batched ScalarE copies land them in an SBUF
    message chunk pool (wrapped token layout, no DRAM round trip).
  - Phase 2 (interleaved per chunk): SBUF-source dma_gather reorders
    message rows into dst-window order (transposed [dout, e] output),
    PE-transpose + batched DVE copy yield Msort rows, then
    matmul(lhsT = IndN, rhs = Msort) accumulates out[slot, dout] per
    window. IndN are host-precomputed norm-weighted indicator tiles
    streamed from DRAM. Bias enters as a K=1 matmul.
  - Layer-1 output rows go straight into the wrapped h1 block layout
    (one 2KB/partition write per 8 windows), AllGather moves the wrapped
    blocks, layer 2 reads them as its feature table. Layer-2 output rows
    are written to the [NB, d] f32 result.
"""

import os
import sys
import numpy as np

for _p in ("/opt/trn_rl_repo", "/root/.axon_site/_ro/trn_rl_repo"):
    if os.path.isdir(_p) and _p not in sys.path:
        sys.path.append(_p)

import ml_dtypes
import concourse.bass as bass
import concourse.mybir as mybir
import concourse.tile as tile
import concourse.bacc as bacc
from concourse.bass_utils import run_bass_kernel_spmd

# --- DMASW lane <- SWDGE queue fix -----------------------------------------
# Tile assigns DMASW semaphore lanes to Pool DMA instructions round-robin in
# scheduled order, independent of their SWDGE queue. Lane-threshold waits are
# only sound if same-lane DMAs complete in order, which holds within one
# SWDGE queue (one ring, FIFO) but not across queues (they run on different
# Q7 core pairs concurrently). Derive the lane from queue_num so every lane
# is fed by exactly one queue.
import concourse.tile_sem_assignment as _tsa
from concourse.tile_scheduler import DMAInst as _DMAInst

_orig_assign_tick = _tsa.TileClockTick._assign_tick


def _assign_tick_qlane(self, inst):
    if (isinstance(inst, _DMAInst)
            and inst.engine == mybir.EngineType.Pool):
        q = getattr(inst, "queue_num", None)
        if q is not None:
            q = int(q)
            cnt = getattr(self, "_qlane_cnt", None)
            if cnt is None:
                cnt = self._qlane_cnt = {}
            k = cnt.get(q, 0)
            cnt[q] = k + 1
            lanes = self.swdge_sem_count
            per = max(1, lanes // 4)
            self.next_sw_dma_idx = (q * per + (k % per)) % lanes
    return _orig_assign_tick(self, inst)


_tsa.TileClockTick._assign_tick = _assign_tick_qlane
# ---------------------------------------------------------------------------

P = 128
HALFT = 25088    # tokens per half of the wrapped table
GB1 = 8          # phase-1 gather batch (tiles per call)
GB2 = 12         # phase-2 gather batch (tiles per call)
CPY = 8          # tiles per batched PSUM->SBUF copy / IndN load
N_CHUNKS = 6
NQ = int(os.environ.get('KNQ', '4'))   # SWDGE queues


def _dma_gather_nt(g, out_ap, in_ap, idxs_ap, num_idxs, elem_size,
                   queue_num, sbuf_tokens_per_rank, sbuf_free_dim_per_rank):
    """SBUF-source dma_gather with transpose=False.

    bass.dma_gather asserts transpose for SBUF sources, but the Q7 ucode's
    descriptor generator handles src_is_sbuf in the non-transpose branch the
    same way it handles HBM sources (only the tx-side addresses differ).
    Non-transpose avoids the shared-XBAR spray path, which corrupts data
    when gathers on different SWDGE queues run concurrently.
    """
    import concourse.ap_utils as ap_utils
    assert idxs_ap.dtype == mybir.dt.int16
    assert in_ap.dtype == out_ap.dtype
    elem_size_bytes = elem_size * mybir.dt.size(in_ap.dtype)
    assert elem_size_bytes % 256 == 0
    assert ap_utils.ap_is_contiguous(in_ap.ap[1:])
    assert ap_utils.ap_is_contiguous(out_ap.ap[1:])
    assert ap_utils.ap_is_contiguous(idxs_ap.ap[1:])
    assert num_idxs % P == 0
    assert out_ap.ap[-1][1] == elem_size
    assert out_ap.ap[0][1] * out_ap.ap[1][1] == num_idxs
    inst = g.add_instruction(
        mybir.InstDMAGatherAnt(
            name=g.bass.get_next_instruction_name(),
            ins=[
                g.lower_ap(in_ap),
                g.lower_ap(idxs_ap),
                g.lower_val_access(g.to_reg(num_idxs)),
            ],
            outs=[g.lower_ap(out_ap)],
            transpose=False,
            num_idxs=num_idxs,
            elem_size=elem_size,
            stride_bytes_256=0,
            gen_mode=0,
            single_packet=False,
            queue_num=queue_num,
            sbuf_tokens_per_rank=sbuf_tokens_per_rank,
            sbuf_free_dim_per_rank=sbuf_free_dim_per_rank,
            sbuf_free_dim_pad_per_rank=0,
            sbuf_byte_offset=0,
        )
    )
    return inst


def _ceil_div(a, b):
    return (a + b - 1) // b


def _wrap_idx16(flat):
    n = len(flat)
    assert n % 16 == 0
    a = np.asarray(flat, np.int16).reshape(n // 16, 16)
    a = np.ascontiguousarray(a.T)
    return np.ascontiguousarray(np.tile(a, (8, 1)))


def _preprocess(src, dst, etype, norm, n_nodes, n_rels, n_cores):
    NB = n_nodes // n_cores
    NW = _ceil_div(NB, P)
    NBP = NW * P                    # padded block rows (6272)
    NE_T = n_rels + 1               # etypes incl. self-loop
    NG = 2 * NE_T                   # (half, etype)

    src = np.asarray(src, np.int64)
    dst = np.asarray(dst, np.int64)
    etype = np.asarray(etype, np.int64)
    norm = np.asarray(norm, np.float32).reshape(-1)

    cores = []
    cnt_gw_all = np.zeros((n_cores, NG, NW), np.int64)
    for c in range(n_cores):
        m = (dst // NB) == c
        es, ed, ee, en = src[m], dst[m], etype[m], norm[m]
        # append self-loop edges
        blk = np.arange(c * NB, (c + 1) * NB, dtype=np.int64)
        es = np.concatenate([es, blk])
        ed = np.concatenate([ed, blk])
        ee = np.concatenate([ee, np.full(NB, n_rels, np.int64)])
        en = np.concatenate([en, np.ones(NB, np.float32)])
        tok = (es // NB) * NBP + (es % NB)
        half = tok // HALFT
        g = half * NE_T + ee
        w = (ed - c * NB) // P
        cnt_gw = np.zeros((NG, NW), np.int64)
        np.add.at(cnt_gw, (g, w), 1)
        cnt_gw_all[c] = cnt_gw
        cores.append((es, ed, ee, en, tok, g, w))

    S_g = cnt_gw_all.sum(2)
    T_g = np.array([_ceil_div(int(S_g[:, g].max()), P) for g in range(NG)])
    base_g = np.concatenate([[0], np.cumsum(T_g)])
    T1 = int(base_g[-1])

    # chunks = contiguous whole groups
    tgt = _ceil_div(T1, N_CHUNKS)
    chunk_of_g = np.zeros(NG, np.int64)
    ck, used = 0, 0
    for g in range(NG):
        if used >= tgt and ck < N_CHUNKS - 1:
            ck, used = ck + 1, 0
        chunk_of_g[g] = ck
        used += T_g[g]
    n_chunks = int(chunk_of_g.max()) + 1
    chunk_t0 = [int(base_g[int(np.argmax(chunk_of_g == k))])
                for k in range(n_chunks)]
    chunk_t1 = chunk_t0[1:] + [T1]
    chunk_tiles = [chunk_t1[k] - chunk_t0[k] for k in range(n_chunks)]

    half_of_tile = np.zeros(T1, np.int64)
    rel_of_tile = np.zeros(T1, np.int64)
    for g in range(NG):
        half_of_tile[base_g[g]:base_g[g + 1]] = g // NE_T
        rel_of_tile[base_g[g]:base_g[g + 1]] = g % NE_T

    p1_calls = []       # (chunk, half, t0, ktiles)
    for k in range(n_chunks):
        t = chunk_t0[k]
        while t < chunk_t1[k]:
            kk = 1
            while (kk < GB1 and t + kk < chunk_t1[k]
                   and half_of_tile[t + kk] == half_of_tile[t]):
                kk += 1
            p1_calls.append((k, int(half_of_tile[t]), t, kk))
            t += kk

    cnt_cw = np.zeros((n_cores, n_chunks, NW), np.int64)
    for c in range(n_cores):
        for g in range(NG):
            cnt_cw[c, chunk_of_g[g]] += cnt_gw_all[c, g]
    T_cw = np.zeros((n_chunks, NW), np.int64)
    for k in range(n_chunks):
        for w in range(NW):
            T_cw[k, w] = _ceil_div(int(cnt_cw[:, k, w].max()), P)
    base_cw = np.concatenate([[0], np.cumsum(T_cw.reshape(-1))])
    T2 = int(base_cw[-1])

    p2_calls = []       # (chunk, t0, ktiles)
    for k in range(n_chunks):
        t0 = int(base_cw[k * NW])
        t1 = int(base_cw[(k + 1) * NW]) if k + 1 < n_chunks else T2
        t = t0
        while t < t1:
            kk = min(GB2, t1 - t)
            p2_calls.append((k, t, kk))
            t += kk

    per_core = []
    for c in range(n_cores):
        es, ed, ee, en, tok, g, w = cores[c]
        ne = len(es)
        cnt_gw = cnt_gw_all[c]

        run_start1 = np.zeros((NG, NW), np.int64)
        for gg in range(NG):
            run_start1[gg] = base_g[gg] * P + np.concatenate(
                [[0], np.cumsum(cnt_gw[gg])[:-1]])
        run_start2 = np.zeros((NG, NW), np.int64)
        for k in range(n_chunks):
            gsel = np.where(chunk_of_g == k)[0]
            for w_ in range(NW):
                p0 = base_cw[k * NW + w_] * P
                run_start2[gsel, w_] = p0 + np.concatenate(
                    [[0], np.cumsum(cnt_gw[gsel, w_])[:-1]])

        order = np.lexsort((w, g))
        gs, ws = g[order], w[order]
        key = gs * NW + ws
        first = np.concatenate([[True], key[1:] != key[:-1]])
        runid = np.cumsum(first) - 1
        firstpos = np.nonzero(first)[0]
        off = np.arange(ne) - firstpos[runid]
        slot1 = np.empty(ne, np.int64)
        slot2 = np.empty(ne, np.int64)
        slot1[order] = run_start1[gs, ws] + off
        slot2[order] = run_start2[gs, ws] + off

        p1_idx = np.zeros(T1 * P, np.int64)
        p1_idx[slot1] = tok - (tok // HALFT) * HALFT

        p2_idx = np.zeros(T2 * P, np.int64)
        ck_of_slot1 = np.searchsorted(
            np.asarray(chunk_t0[1:]) * P, slot1, "right")
        csb = np.asarray([chunk_t0[k] * P for k in range(n_chunks)])
        p2_idx[slot2] = slot1 - csb[ck_of_slot1]

        indn = np.zeros((P, T2 * P), np.float32)
        loc = (ed - c * NB) % P
        indn[slot2 % P, (slot2 // P) * P + loc] = en

        per_core.append(dict(
            p1i=_wrap_idx16(p1_idx),
            p2i=_wrap_idx16(p2_idx),
            indn=np.ascontiguousarray(indn.astype(ml_dtypes.bfloat16)),
        ))

    struct = dict(
        NB=NB, NW=NW, NBP=NBP, T1=T1, T2=T2,
        n_chunks=n_chunks, chunk_t0=[int(x) for x in chunk_t0],
        chunk_tiles=[int(x) for x in chunk_tiles],
        base_cw=[int(x) for x in base_cw],
        T_cw=[[int(T_cw[k, w]) for w in range(NW)] for k in range(n_chunks)],
        rel_of_tile=[int(x) for x in rel_of_tile],
        half_of_tile=[int(x) for x in half_of_tile],
        p1_calls=p1_calls, p2_calls=p2_calls,
        n_rels=n_rels, n_cores=n_cores,
    )
    return struct, per_core


def _build_program(struct, n_nodes, d):
    NB, NW, NBP = struct["NB"], struct["NW"], struct["NBP"]
    T1, T2 = struct["T1"], struct["T2"]
    n_chunks = struct["n_chunks"]
    chunk_t0 = struct["chunk_t0"]
    chunk_tiles = struct["chunk_tiles"]
    base_cw, T_cw = struct["base_cw"], struct["T_cw"]
    rel_of_tile = struct["rel_of_tile"]
    p1_calls, p2_calls = struct["p1_calls"], struct["p2_calls"]
    n_rels = struct["n_rels"]
    n_cores = struct["n_cores"]
    f32, bf16, i16 = mybir.dt.float32, mybir.dt.bfloat16, mybir.dt.int16
    Act = mybir.ActivationFunctionType
    NGW = n_rels + 1
    MCT = max(chunk_tiles)
    HR = HALFT // P                 # ranks per half (196)

    nc = bacc.Bacc("TRN2", target_bir_lowering=False, debug=False,
                   num_devices=n_cores, num_swdge_queues=NQ)

    h0W = nc.dram_tensor("h0W", [P, 2 * HR * d], bf16, kind="ExternalInput")
    w1 = nc.dram_tensor("w1", [P, NGW * d], bf16, kind="ExternalInput")
    w2 = nc.dram_tensor("w2", [P, NGW * d], bf16, kind="ExternalInput")
    b1 = nc.dram_tensor("b1", [1, d], bf16, kind="ExternalInput")
    b2 = nc.dram_tensor("b2", [1, d], bf16, kind="ExternalInput")
    p1i = nc.dram_tensor("p1i", [P, T1 * 8], i16, kind="ExternalInput")
    p2i = nc.dram_tensor("p2i", [P, T2 * 8], i16, kind="ExternalInput")
    indn = nc.dram_tensor("indn", [P, T2 * P], bf16, kind="ExternalInput")
    out = nc.dram_tensor("out", [NB, d], f32, kind="ExternalOutput")

    h1W = nc.dram_tensor("h1W", [P, NW * d], bf16)
    h1Wfull = nc.dram_tensor("h1Wfull", [n_cores * P, NW * d], bf16)

    qctr = [0]

    def next_q():
        q = qctr[0] % NQ
        qctr[0] += 1
        return q

    with tile.TileContext(nc) as tc:
        with (
            tc.tile_pool(name="cst", bufs=1) as cst,
            tc.tile_pool(name="wp", bufs=1) as wp,
            tc.tile_pool(name="hp", bufs=1) as hp,
            tc.tile_pool(name="msp", bufs=1) as msp,
            tc.tile_pool(name="g1p", bufs=6) as g1p,
            tc.tile_pool(name="g2p", bufs=6) as g2p,
            tc.tile_pool(name="mso", bufs=3) as mso,
            tc.tile_pool(name="inp", bufs=3) as inp,
            tc.tile_pool(name="accp", bufs=1) as accp,
            tc.tile_pool(name="wbp", bufs=2) as wbp,
            tc.tile_pool(name="ps1", bufs=2, space="PSUM") as ps1,
            tc.tile_pool(name="pst", bufs=2, space="PSUM") as pst,
            tc.tile_pool(name="pso", bufs=2, space="PSUM") as pso,
        ):
            ident = cst.tile([P, P], bf16)
            nc.gpsimd.memset(ident[:], 0.0)
            nc.gpsimd.affine_select(
                out=ident[:], in_=ident[:],
                compare_op=mybir.AluOpType.not_equal, fill=1.0,
                base=0, pattern=[[-1, P]], channel_multiplier=1,
            )
            ones_row = cst.tile([1, P], bf16)
            nc.gpsimd.memset(ones_row[:], 1.0)
            p1i_sb = cst.tile([P, T1 * 8], i16)
            nc.sync.dma_start(p1i_sb[:], p1i[:, :])
            p2i_sb = cst.tile([P, T2 * 8], i16)
            nc.sync.dma_start(p2i_sb[:], p2i[:, :])
            b1_sb = cst.tile([1, d], bf16)
            nc.sync.dma_start(b1_sb[:], b1[:, :])
            b2_sb = cst.tile([1, d], bf16)
            nc.sync.dma_start(b2_sb[:], b2[:, :])

            def layer(first_layer, w_dram, bias_sb, relu, out_dram):
                w_sb = wp.tile([P, NGW * d], bf16, tag="w")
                nc.sync.dma_start(w_sb[:], w_dram[:, :])

                hh = {}

                def ensure_half(hf):
                    if hf in hh:
                        return hh[hf]
                    ht = hp.tile([P, HR * d], bf16, tag="h")
                    if first_layer:
                        nc.sync.dma_start(
                            ht[:], h0W[:, hf * HR * d:(hf + 1) * HR * d])
                    else:
                        nc.sync.dma_start(
                            ht[:].rearrange("p (c f) -> p c f", f=NW * d),
                            h1Wfull.rearrange("(c p) f -> p c f", p=P)
                            [:, 4 * hf:4 * hf + 4, :])
                    hh.clear()
                    hh[hf] = ht
                    return ht

                accs = [None] * NW
                p1c = [0]
                p2c = [0]

                ind_of_tile = {}

                def ensure_ind(t):
                    t0 = (t // CPY) * CPY
                    if t0 in ind_of_tile:
                        return
                    nt = min(CPY, T2 - t0)
                    it = inp.tile([P, CPY * P], bf16, tag="ind")
                    nc.sync.dma_start(it[:, :nt * P],
                                      indn[:, t0 * P:(t0 + nt) * P])
                    ind_of_tile[t0] = it

                for ck in range(n_chunks):
                    # ---- phase 1 of chunk ck ----
                    msgs = msp.tile([P, MCT * P], bf16, tag="ms")
                    mm1_ps = [None, 0]

                    def flush_mm1(t_end):
                        if mm1_ps[0] is None:
                            return
                        nt = t_end - mm1_ps[1]
                        r0 = (mm1_ps[1] - chunk_t0[ck]) * P
                        nc.scalar.activation(
                            msgs[:, r0:r0 + nt * P], mm1_ps[0][:, :nt * P],
                            Act.Copy)
                        mm1_ps[0] = None

                    while p1c[0] < len(p1_calls) and p1_calls[p1c[0]][0] == ck:
                        _, hf, t0, ktiles = p1_calls[p1c[0]]
                        p1c[0] += 1
                        ht = ensure_half(hf)
                        xt = g1p.tile([P, GB1 * P], bf16, tag="g1")
                        _dma_gather_nt(
                            nc.gpsimd,
                            out_ap=xt[:, :ktiles * P].rearrange(
                                "p (t e) -> p t e", e=d),
                            in_ap=ht[:].rearrange("p (t e) -> p t e", e=d),
                            idxs_ap=p1i_sb[:, t0 * 8:(t0 + ktiles) * 8],
                            num_idxs=ktiles * P, elem_size=d,
                            queue_num=next_q(),
                            sbuf_tokens_per_rank=P,
                            sbuf_free_dim_per_rank=d * 2,
                        )
                        # batched PE transpose of X rows -> X^T columns
                        xtr_ps = pst.tile([P, GB1 * P], bf16, tag="xtr",
                                          space="PSUM", name="xtrps")
                        for kk in range(ktiles):
                            nc.tensor.transpose(
                                out=xtr_ps[:, kk * P:(kk + 1) * P],
                                in_=xt[:, kk * P:(kk + 1) * P],
                                identity=ident[:])
                        xts = mso.tile([P, GB1 * P], bf16, tag="xts")
                        nc.vector.tensor_copy(xts[:, :ktiles * P],
                                              xtr_ps[:, :ktiles * P])
                        for kk in range(ktiles):
                            t = t0 + kk
                            if mm1_ps[0] is not None and t - mm1_ps[1] == CPY:
                                flush_mm1(t)
                            if mm1_ps[0] is None:
                                mm1_ps[0] = ps1.tile([P, CPY * P], f32,
                                                     tag="m1", space="PSUM",
                                                     name="m1ps")
                                mm1_ps[1] = t
                            r = rel_of_tile[t]
                            nc.tensor.matmul(
                                out=mm1_ps[0][:, (t - mm1_ps[1]) * P
                                              :(t - mm1_ps[1] + 1) * P],
                                lhsT=xts[:, kk * P:(kk + 1) * P],
                                rhs=w_sb[:, r * d:(r + 1) * d],
                                start=True, stop=True,
                            )
                        flush_mm1(t0 + ktiles)

                    # ---- phase 2 of chunk ck ----
                    gather_of_tile = {}

                    def ensure_gathered(t):
                        while (t not in gather_of_tile
                               and p2c[0] < len(p2_calls)
                               and p2_calls[p2c[0]][0] == ck):
                            _, t0_, kt_ = p2_calls[p2c[0]]
                            p2c[0] += 1
                            g2 = g2p.tile([P, GB2 * P], bf16, tag="g2")
                            _dma_gather_nt(
                                nc.gpsimd,
                                out_ap=g2[:, :kt_ * P].rearrange(
                                    "p (t e) -> p t e", e=d),
                                in_ap=msgs[:, :chunk_tiles[ck] * P].rearrange(
                                    "p (t e) -> p t e", e=d),
                                idxs_ap=p2i_sb[:, t0_ * 8:(t0_ + kt_) * 8],
                                num_idxs=kt_ * P, elem_size=d,
                                queue_num=next_q(),
                                sbuf_tokens_per_rank=P,
                                sbuf_free_dim_per_rank=d * 2,
                            )
                            for k in range(kt_):
                                gather_of_tile[t0_ + k] = (g2, k)

                    for w in range(NW):
                        ntl = T_cw[ck][w]
                        has_bias = ck == 0
                        if ntl == 0 and not has_bias:
                            continue
                        gbase = int(base_cw[ck * NW + w])
                        o_ps = pso.tile([P, P], f32, tag="op", space="PSUM")
                        n_mm = ntl + (1 if has_bias else 0)
                        i_mm = 0
                        for k in range(ntl):
                            t = gbase + k
                            ensure_gathered(t)
                            g2, gk = gather_of_tile[t]
                            ensure_ind(t)
                            it = ind_of_tile[(t // CPY) * CPY]
                            ik = t - (t // CPY) * CPY
                            nc.tensor.matmul(
                                out=o_ps[:],
                                lhsT=it[:, ik * P:(ik + 1) * P],
                                rhs=g2[:, gk * P:(gk + 1) * P],
                                start=(i_mm == 0),
                                stop=(i_mm == n_mm - 1))
                            i_mm += 1
                        if has_bias:
                            nc.tensor.matmul(
                                out=o_ps[:],
                                lhsT=ones_row[0:1, :],
                                rhs=bias_sb[0:1, :],
                                start=(i_mm == 0), stop=True)
                        if accs[w] is None:
                            acc = accp.tile([P, P], f32, tag=f"acc{w}")
                            nc.vector.tensor_copy(acc[:], o_ps[:])
                            accs[w] = acc
                        else:
                            nc.vector.tensor_add(accs[w][:], accs[w][:],
                                                 o_ps[:])

                # ---- epilogue ----
                WB = 8
                wb = None
                wb_w0 = 0
                for w in range(NW):
                    if wb is None:
                        wb = wbp.tile([P, WB * P],
                                      bf16 if first_layer else f32, tag="wb")
                        wb_w0 = w
                    seg = wb[:, (w - wb_w0) * P:(w - wb_w0 + 1) * P]
                    nc.scalar.activation(seg, accs[w][:],
                                         Act.Relu if relu else Act.Copy)
                    if w - wb_w0 + 1 == WB or w == NW - 1:
                        nw_ = w - wb_w0 + 1
                        if first_layer:
                            # wrapped block layout: row w*128+p at [p, w]
                            nc.sync.dma_start(
                                out_dram[:, wb_w0 * d:(wb_w0 + nw_) * d],
                                wb[:, :nw_ * P])
                        else:
                            rows = min(nw_ * P, NB - wb_w0 * P)
                            full = (rows // P) * P
                            if full:
                                nc.sync.dma_start(
                                    out_dram[wb_w0 * P:wb_w0 * P + full, :]
                                    .rearrange("(t p) d -> p t d", p=P),
                                    wb[:, :full].rearrange(
                                        "p (t d) -> p t d", d=P))
                            rem = rows - full
                            if rem:
                                nc.sync.dma_start(
                                    out_dram[wb_w0 * P + full
                                             :wb_w0 * P + rows, :],
                                    wb[:rem, full:full + P])
                        wb = None

            layer(True, w1, b1_sb, True, h1W)
            nc.gpsimd.collective_compute(
                "AllGather", mybir.AluOpType.bypass,
                replica_groups=[list(range(n_cores))],
                ins=[h1W.ap().opt()], outs=[h1Wfull.ap().opt()],
            )
            layer(False, w2, b2_sb, False, out)

    nc.finalize()
    return nc


_CACHE = {}


def _get_program(struct, n_nodes, d):
    key = (n_nodes, d, struct["T1"], struct["T2"],
           tuple(struct["rel_of_tile"]), tuple(struct["base_cw"]))
    if key not in _CACHE:
        _CACHE[key] = _build_program(struct, n_nodes, d)
    return _CACHE[key]


def prepare(h_ids, src, dst, etype, norm, embedding,
            w_comp1, bases1, loop_w1, bias1,
            w_comp2, bases2, loop_w2, bias2, n_cores=8):
    h_ids = np.asarray(h_ids).astype(np.int64)
    src = np.asarray(src).astype(np.int64)
    dst = np.asarray(dst).astype(np.int64)
    etype = np.asarray(etype).astype(np.int64)
    norm = np.asarray(norm, dtype=np.float32)
    embedding = np.asarray(embedding, dtype=np.float32)
    n_nodes, d = embedding.shape
    n_rels = np.asarray(w_comp1).shape[0]
    NB = n_nodes // n_cores
    NW = _ceil_div(NB, P)
    NBP = NW * P

    W1 = np.einsum("rb,bio->rio", np.asarray(w_comp1, np.float64),
                   np.asarray(bases1, np.float64)).astype(np.float32)
    W2 = np.einsum("rb,bio->rio", np.asarray(w_comp2, np.float64),
                   np.asarray(bases2, np.float64)).astype(np.float32)
    W1 = np.concatenate([W1, np.asarray(loop_w1, np.float32)[None]], 0)
    W2 = np.concatenate([W2, np.asarray(loop_w2, np.float32)[None]], 0)
    w1_dev = np.ascontiguousarray(
        np.transpose(W1, (1, 0, 2)).reshape(d, (n_rels + 1) * d)
    ).astype(ml_dtypes.bfloat16)
    w2_dev = np.ascontiguousarray(
        np.transpose(W2, (1, 0, 2)).reshape(d, (n_rels + 1) * d)
    ).astype(ml_dtypes.bfloat16)
    b1_dev = np.asarray(bias1, np.float32).reshape(1, d).astype(
        ml_dtypes.bfloat16)
    b2_dev = np.asarray(bias2, np.float32).reshape(1, d).astype(
        ml_dtypes.bfloat16)

    h0 = embedding[h_ids].astype(ml_dtypes.bfloat16)
    # wrapped padded-block layout: token c*NBP + r -> [tok%128, tok//128]
    h0pad = np.zeros((n_cores * NBP, d), ml_dtypes.bfloat16)
    for c in range(n_cores):
        h0pad[c * NBP:c * NBP + NB] = h0[c * NB:(c + 1) * NB]
    h0W = np.ascontiguousarray(
        h0pad.reshape(-1, P, d).transpose(1, 0, 2).reshape(P, -1))

    struct, per_core = _preprocess(src, dst, etype, norm, n_nodes, n_rels,
                                   n_cores)

    in_maps = []
    for c in range(n_cores):
        pc = per_core[c]
        in_maps.append({
            "h0W": h0W,
            "w1": w1_dev, "w2": w2_dev, "b1": b1_dev, "b2": b2_dev,
            "p1i": pc["p1i"], "p2i": pc["p2i"], "indn": pc["indn"],
        })
    return struct, in_maps, n_nodes, d


def run(h_ids, src, dst, etype, norm, embedding,
        w_comp1, bases1, loop_w1, bias1,
        w_comp2, bases2, loop_w2, bias2,
        n_cores=8, trace=False):
    struct, in_maps, n_nodes, d = prepare(
        h_ids, src, dst, etype, norm, embedding,
        w_comp1, bases1, loop_w1, bias1,
        w_comp2, bases2, loop_w2, bias2, n_cores)
    nc = _get_program(struct, n_nodes, d)
    res = run_bass_kernel_spmd(
        nc, in_maps, core_ids=list(range(n_cores)), trace=trace)
    blocks = [res.results[c]["out"] for c in range(n_cores)]
    full = np.concatenate(blocks, 0)[:n_nodes]
    if trace:
        return full, res
    return full


def kernel(h_ids, src, dst, etype, norm, embedding,
           w_comp1, bases1, loop_w1, bias1,
           w_comp2, bases2, loop_w2, bias2):
    return run(h_ids, src, dst, etype, norm, embedding,
               w_comp1, bases1, loop_w1, bias1,
               w_comp2, bases2, loop_w2, bias2)
